# revision 26
# baseline (speedup 1.0000x reference)
"""Trainium2 Bass kernel for DifferentiableWeightedRadialFrequencyLoss.

Math:
  loss = sum_{n,c,u,v} Wmap[u,v] * |FFT2(pred-gt)[u,v]|^2 / size
with Wmap = sum_b w_b * mask_b (bands disjoint), in unshifted (ifftshift)
frequency coordinates.

Device algorithm (per core, 12 images = 6 pairs), v3:
  - pack two real images per complex FFT: Z = E1 + i*E2 (Wmap is symmetric
    under (u,v) -> (-u,-v), so cross terms cancel exactly).
  - stage 1 (h-transform): dense o1 = Z^T @ D as 4M complex products that
    accumulate in PSUM (no DVE combines), with fp8e4 operands in DoubleRow
    perf mode (K=256 per matmul: c-chunk pairs in the free axis).  D carries
    a fixed random per-column dither scale (compensated in the weight map)
    to decorrelate fp8 quantization error, plus a global x8 scale.
  - stage 2 (w-transform): radix-4 DIF.  The four natural 128-row blocks of
    o1 are butterflied (L1 on DVE from PSUM, L2 on GPSIMD in SBUF) into
    g_r = sum_m (-i)^{mr} o1_m, then four independent 128-point transforms
    with twiddle-folded bf16 constants N_r[b,y] = w128^{by} w512^{br}/sqrt(512)
    run as 4M complex matmuls (K=128).  Output F[4y+r, u] = (N_r^T g_r)[y,u].
  - power: ACT squares PSUM H-planes, DVE adds + weighted row-sum
    (scalar_tensor_tensor accum_out) against the host-precomputed,
    dither-compensated weight map Wt[y,r,u] = Wu[u,4y+r]/(su[u]^2 s1^2).
Host: shard batch across 8 cores, sum partial accumulators, divide by size.
"""

import numpy as np
import ml_dtypes

import concourse.bass as bass
import concourse.bacc as bacc
import concourse.tile as tile
from concourse import mybir
from concourse.bass_utils import run_bass_kernel_spmd

N_CORES = 8
N, C, H = 32, 3, 512
NUM_BANDS = 16
IMGS_PER_CORE = (N // N_CORES) * C          # 12
PAIRS = IMGS_PER_CORE // 2                  # 6
F32 = mybir.dt.float32
BF16 = mybir.dt.bfloat16
FP8 = mybir.dt.float8e4
ALU = mybir.AluOpType
DRMODE = mybir.MatmulPerfMode.DoubleRow

S1 = 4.0            # global scale on D (folded out via wt)
S2 = 8.0            # global scale on the stage-2 stacks (folded out via wt)
DITHER_SEED = 7     # per-column dither on D / stacks (folded out via wt)

# exposed for test.py introspection
last_results = None
last_nc = None
last_in_maps = None


def _build_nc_v3(repeat=None, nwarm=16, zr_eng="pool", l2_eng="pool",
                 ldbufs=2, zbufs=2):
    """v3: fp8-DoubleRow dense stage 1 + radix-4 DIF bf16 stage 2.

    repeat=N wraps the whole per-core body in a hardware For_i loop for
    steady-state timing (same contract as the baseline builds)."""
    from contextlib import nullcontext
    nc = bacc.Bacc("TRN2", target_bir_lowering=False, debug=False,
                   num_devices=N_CORES)
    pred = nc.dram_tensor("pred", [IMGS_PER_CORE, H, H], F32, kind="ExternalInput")
    gt = nc.dram_tensor("gt", [IMGS_PER_CORE, H, H], F32, kind="ExternalInput")
    d_r = nc.dram_tensor("d_r", [H, H], FP8, kind="ExternalInput")   # Re(D)*s
    d_i = nc.dram_tensor("d_i", [H, H], FP8, kind="ExternalInput")   # Im(D)*s
    d_n = nc.dram_tensor("d_n", [H, H], FP8, kind="ExternalInput")   # -Im(D)*s
    n_r = nc.dram_tensor("n_r", [128, 4, 128], BF16, kind="ExternalInput")
    n_i = nc.dram_tensor("n_i", [128, 4, 128], BF16, kind="ExternalInput")
    n_n = nc.dram_tensor("n_n", [128, 4, 128], BF16, kind="ExternalInput")
    wt = nc.dram_tensor("wt", [128, 4, H], BF16, kind="ExternalInput")
    out = nc.dram_tensor("out", [128, PAIRS * 4], F32, kind="ExternalOutput")

    def r4(ap):  # [512, 512] dram view -> [128 part, 4 chunks, 512]
        return ap.rearrange("(c p) w -> p c w", p=128)

    with tile.TileContext(nc) as tc:
        with (
            tc.tile_pool(name="consts", bufs=1) as consts,
            tc.tile_pool(name="loads", bufs=ldbufs) as loads,
            tc.tile_pool(name="zpool", bufs=zbufs) as zpool,
            tc.tile_pool(name="bpool", bufs=2) as bpool,
            tc.tile_pool(name="gpool", bufs=2) as gpool,
            tc.tile_pool(name="spool", bufs=2) as spool,
            tc.tile_pool(name="ps1", bufs=1, space="PSUM") as ps1,
            tc.tile_pool(name="ps2", bufs=2, space="PSUM") as ps2,
        ):
            dr_sb = consts.tile([128, 4, H], FP8)
            di_sb = consts.tile([128, 4, H], FP8)
            dn_sb = consts.tile([128, 4, H], FP8)
            nr_sb = consts.tile([128, 4, 128], BF16)
            ni_sb = consts.tile([128, 4, 128], BF16)
            nn_sb = consts.tile([128, 4, 128], BF16)
            wt_sb = consts.tile([128, 4, H], BF16)
            acc = consts.tile([128, PAIRS * 4], F32)
            # warmup: keep PE busy during the DMA lead-in so the HAM
            # clock-gate is at full rate when real matmuls start.
            warm = consts.tile([128, H], BF16)
            nc.vector.memset(warm[:], 0.0)
            wps = ps2.tile([128, H], F32, tag="hr")
            for i in range(nwarm):
                nc.tensor.matmul(wps[:], warm[:, 0:128], warm[:],
                                 start=(i == 0), stop=(i == nwarm - 1))

            zre = nc.gpsimd if zr_eng == "pool" else nc.vector
            l2e = nc.gpsimd if l2_eng == "pool" else nc.vector

            def stage2(g, pr):
                # 4 independent 128-point transforms + weighted power
                for r in range(4):
                    grr, gri = g[2 * r], g[2 * r + 1]
                    hr = ps2.tile([128, H], F32, tag="hr")
                    hi = ps2.tile([128, H], F32, tag="hi")
                    nc.tensor.matmul(hr[:], nr_sb[:, r, :], grr[:],
                                     start=True, stop=False)
                    nc.tensor.matmul(hr[:], nn_sb[:, r, :], gri[:],
                                     start=False, stop=True)
                    nc.tensor.matmul(hi[:], ni_sb[:, r, :], grr[:],
                                     start=True, stop=False)
                    nc.tensor.matmul(hi[:], nr_sb[:, r, :], gri[:],
                                     start=False, stop=True)
                    prt = spool.tile([128, H], BF16, tag="prt")
                    pit = spool.tile([128, H], BF16, tag="pit")
                    nc.scalar.square(prt[:], hr[:])
                    nc.scalar.square(pit[:], hi[:])
                    t = spool.tile([128, H], BF16, tag="t")
                    nc.vector.tensor_add(t[:], prt[:], pit[:])
                    gs = spool.tile([128, H], BF16, tag="gs")
                    col = pr * 4 + r
                    nc.vector.scalar_tensor_tensor(
                        out=gs[:], in0=t[:], scalar=0.0, in1=wt_sb[:, r, :],
                        op0=ALU.bypass, op1=ALU.mult,
                        accum_out=acc[:, col: col + 1])

            def half(zr, zi, mA, mB, tagp):
                # stage-1 blocks mA, mB: o1_m = (Z^T D)[128m:128(m+1), :]
                # via 4M fp8 DoubleRow (c-chunk pairs), then L1 butterfly
                # s = o1_mA + o1_mB, d = o1_mA - o1_mB.
                par = ps1.tile([128, H], F32, tag="par")
                pai = ps1.tile([128, H], F32, tag="pai")
                pbr = ps1.tile([128, H], F32, tag="pbr")
                pbi = ps1.tile([128, H], F32, tag="pbi")
                for ps, m in ((par, mA), (pbr, mB)):
                    sl = slice(m * 128, (m + 1) * 128)
                    first = True
                    for cp in (0, 2):
                        cs = slice(cp, cp + 2)
                        nc.tensor.matmul(ps[:], zr[:, cs, sl], dr_sb[:, cs, :],
                                         start=first, stop=False,
                                         perf_mode=DRMODE)
                        nc.tensor.matmul(ps[:], zi[:, cs, sl], dn_sb[:, cs, :],
                                         start=False, stop=(cp == 2),
                                         perf_mode=DRMODE)
                        first = False
                for ps, m in ((pai, mA), (pbi, mB)):
                    sl = slice(m * 128, (m + 1) * 128)
                    first = True
                    for cp in (0, 2):
                        cs = slice(cp, cp + 2)
                        nc.tensor.matmul(ps[:], zr[:, cs, sl], di_sb[:, cs, :],
                                         start=first, stop=False,
                                         perf_mode=DRMODE)
                        nc.tensor.matmul(ps[:], zi[:, cs, sl], dr_sb[:, cs, :],
                                         start=False, stop=(cp == 2),
                                         perf_mode=DRMODE)
                        first = False
                car = bpool.tile([128, H], BF16, tag=f"c{tagp}r")
                cai = bpool.tile([128, H], BF16, tag=f"c{tagp}i")
                nc.scalar.copy(car[:], par[:])
                nc.scalar.copy(cai[:], pai[:])
                sr = bpool.tile([128, H], BF16, tag=f"s{tagp}r")
                si = bpool.tile([128, H], BF16, tag=f"s{tagp}i")
                dr_ = bpool.tile([128, H], BF16, tag=f"d{tagp}r")
                di_ = bpool.tile([128, H], BF16, tag=f"d{tagp}i")
                nc.vector.tensor_add(sr[:], car[:], pbr[:])
                nc.vector.tensor_sub(dr_[:], car[:], pbr[:])
                nc.vector.tensor_add(si[:], cai[:], pbi[:])
                nc.vector.tensor_sub(di_[:], cai[:], pbi[:])
                return sr, si, dr_, di_

            rep_ctx = (
                tc.For_i(0, repeat, 1,
                         hint_engines=(mybir.EngineType.PE,
                                       mybir.EngineType.DVE))
                if repeat is not None else nullcontext()
            )
            with rep_ctx:
              pending = None
              for pr in range(PAIRS):
                i1, i2 = 2 * pr, 2 * pr + 1
                zr = zpool.tile([128, 4, H], FP8, tag="zr")
                zi = zpool.tile([128, 4, H], FP8, tag="zi")
                for c in range(4):
                    p1c = loads.tile([128, H], F32, tag=f"p1t{c}")
                    g1c = loads.tile([128, H], F32, tag=f"g1t{c}")
                    p2c = loads.tile([128, H], F32, tag=f"p2t{c}")
                    g2c = loads.tile([128, H], F32, tag=f"g2t{c}")
                    nc.sync.dma_start(out=p1c[:], in_=r4(pred.ap()[i1])[:, c, :])
                    nc.sync.dma_start(out=g1c[:], in_=r4(gt.ap()[i1])[:, c, :])
                    nc.sync.dma_start(out=p2c[:], in_=r4(pred.ap()[i2])[:, c, :])
                    nc.sync.dma_start(out=g2c[:], in_=r4(gt.ap()[i2])[:, c, :])
                    if pr == 0:
                        # interleave const DMAs between data chunks
                        if c == 0:
                            nc.sync.dma_start(out=dr_sb[:], in_=r4(d_r.ap()))
                            nc.sync.dma_start(out=di_sb[:], in_=r4(d_i.ap()))
                        elif c == 1:
                            nc.sync.dma_start(out=dn_sb[:], in_=r4(d_n.ap()))
                            nc.sync.dma_start(out=nr_sb[:], in_=n_r.ap())
                        elif c == 2:
                            nc.sync.dma_start(out=ni_sb[:], in_=n_i.ap())
                            nc.sync.dma_start(out=nn_sb[:], in_=n_n.ap())
                        else:
                            nc.sync.dma_start(out=wt_sb[:], in_=wt.ap())
                    zre.tensor_sub(zr[:, c, :], p1c[:], g1c[:])
                    nc.vector.tensor_sub(zi[:, c, :], p2c[:], g2c[:])

                s02r, s02i, d02r, d02i = half(zr, zi, 0, 2, "a")
                if pending is not None:
                    stage2(*pending)
                s13r, s13i, d13r, d13i = half(zr, zi, 1, 3, "b")

                # L2 butterfly: g_r = sum_m (-i)^{mr} o1_m
                g = [gpool.tile([128, H], BF16, tag=f"g{k}", name=f"g{k}")
                     for k in range(8)]
                l2e.tensor_add(g[0][:], s02r[:], s13r[:])   # g0r
                l2e.tensor_add(g[1][:], s02i[:], s13i[:])   # g0i
                l2e.tensor_add(g[2][:], d02r[:], d13i[:])   # g1r
                l2e.tensor_sub(g[3][:], d02i[:], d13r[:])   # g1i
                l2e.tensor_sub(g[4][:], s02r[:], s13r[:])   # g2r
                l2e.tensor_sub(g[5][:], s02i[:], s13i[:])   # g2i
                l2e.tensor_sub(g[6][:], d02r[:], d13i[:])   # g3r
                l2e.tensor_add(g[7][:], d02i[:], d13r[:])   # g3i
                pending = (g, pr)
              stage2(*pending)

              nc.sync.dma_start(out=out.ap(), in_=acc[:])

    nc.compile()
    return nc


def _build_nc_v4(repeat=None, nwarm=16, subs="eng", ldchunks=2,
                 loop_warm=0, tail_chunk=True, t_eng="dve", sub_eng="mix", psb=2,
                 l1d_eng="dve"):
    """v4: DMA-accum subtraction (no engine sub ops), fp8-DR dense stage 1,
    stage 2 radix-4 DIF with BOTH butterfly levels folded:
      L1 (s/d of block pairs) on DVE from PSUM (4 batched ops/pair),
      L2 folded into 12 DoubleRow constant stacks (16 DR-MMs/pair).
    Per-pair engine work: PE 48 DR-MMs; DVE 4 L1 + 2 t + 2 stt;
    ACT 2 copies + 4 squares; Pool only SWDGE descriptor generation."""
    from contextlib import nullcontext
    nc = bacc.Bacc("TRN2", target_bir_lowering=False, debug=False,
                   num_devices=N_CORES)
    pred = nc.dram_tensor("pred", [IMGS_PER_CORE, H, H], F32, kind="ExternalInput")
    gt = nc.dram_tensor("gt", [IMGS_PER_CORE, H, H], F32, kind="ExternalInput")
    d_r = nc.dram_tensor("d_r", [H, H], FP8, kind="ExternalInput")
    d_i = nc.dram_tensor("d_i", [H, H], FP8, kind="ExternalInput")
    d_n = nc.dram_tensor("d_n", [H, H], FP8, kind="ExternalInput")
    st = nc.dram_tensor("st", [128, 12, 2, 128], FP8, kind="ExternalInput")
    wt = nc.dram_tensor("wt", [128, 4, H], BF16, kind="ExternalInput")
    out = nc.dram_tensor("out", [128, PAIRS * 2 + 2], F32, kind="ExternalOutput")

    def r4(ap):  # [512, 512] dram view -> [128 part, 4 chunks, 512]
        return ap.rearrange("(c p) w -> p c w", p=128)

    with tile.TileContext(nc) as tc:
        with (
            tc.tile_pool(name="consts", bufs=1) as consts,
            tc.tile_pool(name="loads", bufs=2) as loads,
            tc.tile_pool(name="zpool", bufs=2) as zpool,
            tc.tile_pool(name="sdpool", bufs=2) as sdpool,
            tc.tile_pool(name="cpool", bufs=2) as cpool,
            tc.tile_pool(name="spool", bufs=2) as spool,
            tc.tile_pool(name="ps1", bufs=1, space="PSUM") as ps1,
            tc.tile_pool(name="ps1b", bufs=(2 if psb == 2 else 1),
                         space="PSUM") as ps1b,
            tc.tile_pool(name="ps2", bufs=(1 if psb == 2 else 2),
                         space="PSUM") as ps2,
        ):
            dr_sb = consts.tile([128, 4, H], FP8)
            di_sb = consts.tile([128, 4, H], FP8)
            dn_sb = consts.tile([128, 4, H], FP8)
            st_sb = consts.tile([128, 12, 2, 128], FP8)
            wt_sb = consts.tile([128, 4, H], BF16)
            acc = consts.tile([128, PAIRS * 2 + 2], F32)
            warm = consts.tile([128, H], BF16)
            nc.vector.memset(warm[:], 0.0)
            if psb == 2:
                wps = ps2.tile([128, H], F32, tag="hr")
                wv = wps[:]
            else:
                wps = ps2.tile([128, 2, H], F32, tag="h2")
                wv = wps[:, 0, :]
            for i in range(nwarm):
                nc.tensor.matmul(wv, warm[:, 0:128], warm[:],
                                 start=(i == 0), stop=(i == nwarm - 1))

            def stage2rp(sd, pr, rp, chunked=False, t_eng="dve"):
                # H_r = N_r^T g_r with the L2 butterfly folded into the
                # stacks; per r: Hr = A^T pl_r + B^T pl_i ; Hi = C^T pl_r
                # + A^T pl_i (pl = (P0,P2) for r even, (P1,P3) for r odd).
                te = nc.gpsimd if t_eng == "pool" else nc.vector
                if True:
                    pq = spool.tile([128, 2, 2, H], BF16, tag="pq")
                    for rr in range(2):
                        r = rp * 2 + rr
                        sdsel = 0 if r in (0, 2) else 1
                        ka = (r * 3 + 0)
                        kb = (r * 3 + 1)
                        kc = (r * 3 + 2)
                        rpair = sd[:, sdsel, :, 0, :]
                        ipair = sd[:, sdsel, :, 1, :]
                        if psb == 2:
                            hr = ps2.tile([128, H], F32, tag="hr")
                            hi = ps2.tile([128, H], F32, tag="hi")
                            h0, h1 = hr[:], hi[:]
                        else:
                            h2 = ps2.tile([128, 2, H], F32, tag="h2")
                            h0, h1 = h2[:, 0, :], h2[:, 1, :]
                        nc.tensor.matmul(h0, st_sb[:, ka, :, :],
                                         rpair, start=True, stop=False,
                                         perf_mode=DRMODE)
                        nc.tensor.matmul(h0, st_sb[:, kb, :, :],
                                         ipair, start=False, stop=True,
                                         perf_mode=DRMODE)
                        nc.tensor.matmul(h1, st_sb[:, kc, :, :],
                                         rpair, start=True, stop=False,
                                         perf_mode=DRMODE)
                        nc.tensor.matmul(h1, st_sb[:, ka, :, :],
                                         ipair, start=False, stop=True,
                                         perf_mode=DRMODE)
                        if psb == 2:
                            nc.scalar.square(pq[:, rr, 0, :], h0)
                            nc.scalar.square(pq[:, rr, 1, :], h1)
                        else:
                            nc.scalar.square(pq[:, rr, :, :], h2[:])
                        if chunked:
                            # last-pair tail shrink: per-r weighted accum
                            t1 = spool.tile([128, H], BF16, tag="t1")
                            te.tensor_add(t1[:], pq[:, rr, 0, :],
                                          pq[:, rr, 1, :])
                            g1 = spool.tile([128, H], BF16, tag="g1")
                            col = pr * 2 + rp * 2 + rr
                            nc.vector.scalar_tensor_tensor(
                                out=g1[:], in0=t1[:], scalar=0.0,
                                in1=wt_sb[:, r, :],
                                op0=ALU.bypass, op1=ALU.mult,
                                accum_out=acc[:, col: col + 1])
                    if not chunked:
                        t = spool.tile([128, 2, H], BF16, tag="t")
                        te.tensor_add(t[:], pq[:, :, 0, :], pq[:, :, 1, :])
                        gs = spool.tile([128, 2, H], BF16, tag="gs")
                        col = pr * 2 + rp
                        nc.vector.scalar_tensor_tensor(
                            out=gs[:], in0=t[:], scalar=0.0,
                            in1=wt_sb[:, rp * 2: rp * 2 + 2, :],
                            op0=ALU.bypass, op1=ALU.mult,
                            accum_out=acc[:, col: col + 1])

            def half(zr, zi, mA, mB, sd, hf):
                # stage-1 blocks mA, mB (4M fp8 DoubleRow, c-chunk pairs),
                # then L1 butterfly into sd[:, {s,d}, hf, :, :].
                pa = ps1.tile([128, 2, H], F32, tag="pa")
                pb = ps1b.tile([128, 2, H], F32, tag="pb")
                for ps, m in ((pa, mA), (pb, mB)):
                    sl = slice(m * 128, (m + 1) * 128)
                    for comp, (dc1, dc2) in enumerate(((dr_sb, dn_sb),
                                                       (di_sb, dr_sb))):
                        first = True
                        for cp in (0, 2):
                            cs = slice(cp, cp + 2)
                            nc.tensor.matmul(ps[:, comp, :], zr[:, cs, sl],
                                             dc1[:, cs, :], start=first,
                                             stop=False, perf_mode=DRMODE)
                            nc.tensor.matmul(ps[:, comp, :], zi[:, cs, sl],
                                             dc2[:, cs, :], start=False,
                                             stop=(cp == 2), perf_mode=DRMODE)
                            first = False
                ca = cpool.tile([128, 2, H], BF16, tag="ca")
                if l1d_eng == "pool":
                    # ca2 = 2*o1_A (ACT); s = 0.5*ca2 + o1_B (DVE STT);
                    # d = o1_A - o1_B = ca2 - s on GPSIMD (SBUF-only).
                    # (sign of d is irrelevant: d-planes enter |H|^2
                    # through linear maps only)
                    nc.scalar.mul(ca[:], pa[:], 2.0)
                    sv = sd[:, 0, hf, :, :]
                    nc.vector.scalar_tensor_tensor(
                        out=sv, in0=ca[:], scalar=0.5, in1=pb[:],
                        op0=ALU.mult, op1=ALU.add)
                    nc.gpsimd.tensor_sub(sd[:, 1, hf, :, :], ca[:], sv)
                else:
                    nc.scalar.copy(ca[:], pa[:])
                    nc.vector.tensor_add(sd[:, 0, hf, :, :], ca[:], pb[:])
                    nc.vector.tensor_sub(sd[:, 1, hf, :, :], ca[:], pb[:])

            rep_ctx = (
                tc.For_i(0, repeat, 1,
                         hint_engines=(mybir.EngineType.PE,
                                       mybir.EngineType.DVE))
                if repeat is not None else nullcontext()
            )
            with rep_ctx:
              pending = None
              for pr in range(PAIRS):
                i1, i2 = 2 * pr, 2 * pr + 1
                zr = zpool.tile([128, 4, H], FP8, tag="zr")
                zi = zpool.tile([128, 4, H], FP8, tag="zi")
                if subs == "dma":
                    # z = pred + (-gt) via DMA: cast-load -gt, then
                    # accum-add pred (SWDGE: out = in + out), per c-half to
                    # align with the DoubleRow c-pair consumption.
                    for ch in (0, 2):
                        cs = slice(ch, ch + 2)
                        nc.gpsimd.dma_start(out=zr[:, cs, :],
                                            in_=r4(gt.ap()[i1])[:, cs, :])
                        nc.gpsimd.dma_start(out=zr[:, cs, :],
                                            in_=r4(pred.ap()[i1])[:, cs, :],
                                            accum_op=ALU.add)
                        nc.gpsimd.dma_start(out=zi[:, cs, :],
                                            in_=r4(gt.ap()[i2])[:, cs, :])
                        nc.gpsimd.dma_start(out=zi[:, cs, :],
                                            in_=r4(pred.ap()[i2])[:, cs, :],
                                            accum_op=ALU.add)
                else:
                    # HWDGE chunked loads + engine adds (gt pre-negated on
                    # host): zr on GPSIMD, zi on DVE, per c-half.
                    for ch in (0, 2):
                        cs = slice(ch, ch + 2)
                        p1h = loads.tile([128, 2, H], F32, tag=f"p1h{ch}")
                        g1h = loads.tile([128, 2, H], F32, tag=f"g1h{ch}")
                        p2h = loads.tile([128, 2, H], F32, tag=f"p2h{ch}")
                        g2h = loads.tile([128, 2, H], F32, tag=f"g2h{ch}")
                        nc.sync.dma_start(out=p1h[:], in_=r4(pred.ap()[i1])[:, cs, :])
                        nc.sync.dma_start(out=g1h[:], in_=r4(gt.ap()[i1])[:, cs, :])
                        nc.sync.dma_start(out=p2h[:], in_=r4(pred.ap()[i2])[:, cs, :])
                        nc.sync.dma_start(out=g2h[:], in_=r4(gt.ap()[i2])[:, cs, :])
                        nc.gpsimd.tensor_add(zr[:, cs, :], p1h[:], g1h[:])
                        ze = nc.gpsimd if sub_eng == "pool" else nc.vector
                        ze.tensor_add(zi[:, cs, :], p2h[:], g2h[:])
                if pr == 0:
                    nc.sync.dma_start(out=dr_sb[:], in_=r4(d_r.ap()))
                    nc.sync.dma_start(out=di_sb[:], in_=r4(d_i.ap()))
                    nc.sync.dma_start(out=dn_sb[:], in_=r4(d_n.ap()))
                    nc.sync.dma_start(out=st_sb[:], in_=st.ap())
                    nc.sync.dma_start(out=wt_sb[:], in_=wt.ap())

                if pr == 0 and loop_warm:
                    # keep the HAM clock-gate warm through the load head
                    wps2 = ps1.tile([128, 2, H], F32, tag="pa")
                    for i in range(loop_warm):
                        nc.tensor.matmul(wps2[:, 0, :], warm[:, 0:128],
                                         warm[:], start=(i == 0),
                                         stop=(i == loop_warm - 1))
                sd = sdpool.tile([128, 2, 2, 2, H], FP8, tag="sd")
                half(zr, zi, 0, 2, sd, 0)
                if pending is not None:
                    stage2rp(*pending, 0, t_eng=t_eng)
                half(zr, zi, 1, 3, sd, 1)
                if pending is not None:
                    stage2rp(*pending, 1, t_eng=t_eng)
                pending = (sd, pr)
              stage2rp(*pending, 0, chunked=tail_chunk, t_eng=t_eng)
              stage2rp(*pending, 1, chunked=tail_chunk, t_eng=t_eng)

              nc.sync.dma_start(out=out.ap(), in_=acc[:])

    nc.compile()
    return nc


BUILD = _build_nc_v4


def _host_constants():
    """Precompute the device constant tensors (shared across cores)."""
    e4 = ml_dtypes.float8_e4m3
    rng = np.random.default_rng(DITHER_SEED)
    su = np.exp(rng.uniform(0.0, np.log(2.0), H))        # stage-1 col dither
    sq = np.exp(rng.uniform(0.0, np.log(2.0), 128))      # stage-2 col dither
    j = np.arange(H, dtype=np.float64)
    ang = 2.0 * np.pi * np.outer(j, j) / H
    scale = 1.0 / np.sqrt(H)
    Dre = np.cos(ang) * scale
    Dim = -np.sin(ang) * scale
    drb = np.ascontiguousarray((Dre * S1 * su[None, :]).astype(np.float32).astype(e4))
    dib = np.ascontiguousarray((Dim * S1 * su[None, :]).astype(np.float32).astype(e4))
    dnb = np.ascontiguousarray((-Dim * S1 * su[None, :]).astype(np.float32).astype(e4))
    # stage-2 stacks: quantize N_r first, then build stacks from the
    # quantized planes (so A/B/C/D share bit-identical entries).
    b = np.arange(128, dtype=np.float64)
    y = np.arange(128, dtype=np.float64)
    base = 2.0 * np.pi * np.outer(b, y) / 128.0
    Nq = []
    for r in range(4):
        a = base + (2.0 * np.pi * b * r / H)[:, None]
        m_r = np.cos(a) * scale * S2 * sq[None, :]
        m_i = -np.sin(a) * scale * S2 * sq[None, :]
        Nq.append((m_r.astype(np.float32).astype(e4).astype(np.float32),
                   m_i.astype(np.float32).astype(e4).astype(np.float32)))
    sth = np.empty((128, 12, 2, 128), np.float32)
    for r, (Nr, Ni) in enumerate(Nq):
        if r == 0:
            stacks = [(Nr, Nr), (-Ni, -Ni), (Ni, Ni)]
        elif r == 1:
            stacks = [(Nr, Ni), (-Ni, Nr), (Ni, -Nr)]
        elif r == 2:
            stacks = [(Nr, -Nr), (-Ni, Ni), (Ni, -Ni)]
        else:
            stacks = [(Nr, -Ni), (-Ni, -Nr), (Ni, Nr)]
        for kk, (m0, m1) in enumerate(stacks):
            sth[:, r * 3 + kk, 0, :] = m0
            sth[:, r * 3 + kk, 1, :] = m1
    stb = np.ascontiguousarray(sth.astype(e4))
    return su, sq, drb, dib, dnb, stb


def kernel(predictions, ground_truths, band_weights, band_masks):
    global last_results, last_nc, last_in_maps
    pred = np.ascontiguousarray(np.asarray(predictions, dtype=np.float32))
    gt = np.ascontiguousarray(np.asarray(ground_truths, dtype=np.float32))
    bw = np.asarray(band_weights, dtype=np.float64)
    bm = np.asarray(band_masks, dtype=np.float64)

    wmap = np.einsum('b,bhw->hw', bw, bm)          # shifted coords
    wu = np.fft.ifftshift(wmap)                     # [u_h, v_w] unshifted
    su, sq, drb, dib, dnb, stb = _host_constants()
    # weight tile Wt[y, r, u] = Wu[u, 4y+r] / (su[u]^2 sq[y]^2 S1^2 S2^2)
    yv = np.arange(128)
    wth = np.empty((128, 4, H), np.float64)
    for r in range(4):
        wth[:, r, :] = wu[:, 4 * yv + r].T
    wth /= (su[None, None, :] ** 2) * (sq[:, None, None] ** 2) \
        * (S1 * S1 * S2 * S2)
    bf = ml_dtypes.bfloat16
    wtb = np.ascontiguousarray(wth.astype(np.float32).astype(bf))

    pred_r = pred.reshape(N_CORES, IMGS_PER_CORE, H, H)
    gt_r = gt.reshape(N_CORES, IMGS_PER_CORE, H, H)
    in_maps = [
        {
            "pred": np.ascontiguousarray(pred_r[c]),
            "gt": np.ascontiguousarray(-gt_r[c]),
            "d_r": drb, "d_i": dib, "d_n": dnb,
            "st": stb, "wt": wtb,
        }
        for c in range(N_CORES)
    ]

    nc = BUILD()
    last_nc, last_in_maps = nc, in_maps
    res = run_bass_kernel_spmd(nc, in_maps, core_ids=list(range(N_CORES)))
    last_results = res
    total = np.float64(0.0)
    for r in res.results:
        total += r["out"].astype(np.float64).sum()
    loss = total / float(N * C * H * H)
    return np.float32(loss)


# revision 27
# speedup vs baseline: 1.1979x; 1.1979x over previous
"""Trainium2 Bass kernel for DifferentiableWeightedRadialFrequencyLoss.

Math:
  loss = sum_{n,c,u,v} Wmap[u,v] * |FFT2(pred-gt)[u,v]|^2 / size
with Wmap = sum_b w_b * mask_b (bands disjoint), in unshifted (ifftshift)
frequency coordinates.

Device algorithm (per core, 12 images = 6 pairs), v3:
  - pack two real images per complex FFT: Z = E1 + i*E2 (Wmap is symmetric
    under (u,v) -> (-u,-v), so cross terms cancel exactly).
  - stage 1 (h-transform): dense o1 = Z^T @ D as 4M complex products that
    accumulate in PSUM (no DVE combines), with fp8e4 operands in DoubleRow
    perf mode (K=256 per matmul: c-chunk pairs in the free axis).  D carries
    a fixed random per-column dither scale (compensated in the weight map)
    to decorrelate fp8 quantization error, plus a global x8 scale.
  - stage 2 (w-transform): radix-4 DIF.  The four natural 128-row blocks of
    o1 are butterflied (L1 on DVE from PSUM, L2 on GPSIMD in SBUF) into
    g_r = sum_m (-i)^{mr} o1_m, then four independent 128-point transforms
    with twiddle-folded bf16 constants N_r[b,y] = w128^{by} w512^{br}/sqrt(512)
    run as 4M complex matmuls (K=128).  Output F[4y+r, u] = (N_r^T g_r)[y,u].
  - power: ACT squares PSUM H-planes, DVE adds + weighted row-sum
    (scalar_tensor_tensor accum_out) against the host-precomputed,
    dither-compensated weight map Wt[y,r,u] = Wu[u,4y+r]/(su[u]^2 s1^2).
Host: shard batch across 8 cores, sum partial accumulators, divide by size.
"""

import numpy as np
import ml_dtypes

import concourse.bass as bass
import concourse.bacc as bacc
import concourse.tile as tile
from concourse import mybir
from concourse.bass_utils import run_bass_kernel_spmd

N_CORES = 8
N, C, H = 32, 3, 512
NUM_BANDS = 16
IMGS_PER_CORE = (N // N_CORES) * C          # 12
PAIRS = IMGS_PER_CORE // 2                  # 6
F32 = mybir.dt.float32
BF16 = mybir.dt.bfloat16
FP8 = mybir.dt.float8e4
ALU = mybir.AluOpType
DRMODE = mybir.MatmulPerfMode.DoubleRow

S1 = 4.0            # global scale on D (folded out via wt)
S2 = 8.0            # global scale on the stage-2 stacks (folded out via wt)
DITHER_SEED = 7     # per-column dither on D / stacks (folded out via wt)

# exposed for test.py introspection
last_results = None
last_nc = None
last_in_maps = None


def _build_nc_v3(repeat=None, nwarm=16, zr_eng="pool", l2_eng="pool",
                 ldbufs=2, zbufs=2):
    """v3: fp8-DoubleRow dense stage 1 + radix-4 DIF bf16 stage 2.

    repeat=N wraps the whole per-core body in a hardware For_i loop for
    steady-state timing (same contract as the baseline builds)."""
    from contextlib import nullcontext
    nc = bacc.Bacc("TRN2", target_bir_lowering=False, debug=False,
                   num_devices=N_CORES)
    pred = nc.dram_tensor("pred", [IMGS_PER_CORE, H, H], F32, kind="ExternalInput")
    gt = nc.dram_tensor("gt", [IMGS_PER_CORE, H, H], F32, kind="ExternalInput")
    d_r = nc.dram_tensor("d_r", [H, H], FP8, kind="ExternalInput")   # Re(D)*s
    d_i = nc.dram_tensor("d_i", [H, H], FP8, kind="ExternalInput")   # Im(D)*s
    d_n = nc.dram_tensor("d_n", [H, H], FP8, kind="ExternalInput")   # -Im(D)*s
    n_r = nc.dram_tensor("n_r", [128, 4, 128], BF16, kind="ExternalInput")
    n_i = nc.dram_tensor("n_i", [128, 4, 128], BF16, kind="ExternalInput")
    n_n = nc.dram_tensor("n_n", [128, 4, 128], BF16, kind="ExternalInput")
    wt = nc.dram_tensor("wt", [128, 4, H], BF16, kind="ExternalInput")
    out = nc.dram_tensor("out", [128, PAIRS * 4], F32, kind="ExternalOutput")

    def r4(ap):  # [512, 512] dram view -> [128 part, 4 chunks, 512]
        return ap.rearrange("(c p) w -> p c w", p=128)

    with tile.TileContext(nc) as tc:
        with (
            tc.tile_pool(name="consts", bufs=1) as consts,
            tc.tile_pool(name="loads", bufs=ldbufs) as loads,
            tc.tile_pool(name="zpool", bufs=zbufs) as zpool,
            tc.tile_pool(name="bpool", bufs=2) as bpool,
            tc.tile_pool(name="gpool", bufs=2) as gpool,
            tc.tile_pool(name="spool", bufs=2) as spool,
            tc.tile_pool(name="ps1", bufs=1, space="PSUM") as ps1,
            tc.tile_pool(name="ps2", bufs=2, space="PSUM") as ps2,
        ):
            dr_sb = consts.tile([128, 4, H], FP8)
            di_sb = consts.tile([128, 4, H], FP8)
            dn_sb = consts.tile([128, 4, H], FP8)
            nr_sb = consts.tile([128, 4, 128], BF16)
            ni_sb = consts.tile([128, 4, 128], BF16)
            nn_sb = consts.tile([128, 4, 128], BF16)
            wt_sb = consts.tile([128, 4, H], BF16)
            acc = consts.tile([128, PAIRS * 4], F32)
            # warmup: keep PE busy during the DMA lead-in so the HAM
            # clock-gate is at full rate when real matmuls start.
            warm = consts.tile([128, H], BF16)
            nc.vector.memset(warm[:], 0.0)
            wps = ps2.tile([128, H], F32, tag="hr")
            for i in range(nwarm):
                nc.tensor.matmul(wps[:], warm[:, 0:128], warm[:],
                                 start=(i == 0), stop=(i == nwarm - 1))

            zre = nc.gpsimd if zr_eng == "pool" else nc.vector
            l2e = nc.gpsimd if l2_eng == "pool" else nc.vector

            def stage2(g, pr):
                # 4 independent 128-point transforms + weighted power
                for r in range(4):
                    grr, gri = g[2 * r], g[2 * r + 1]
                    hr = ps2.tile([128, H], F32, tag="hr")
                    hi = ps2.tile([128, H], F32, tag="hi")
                    nc.tensor.matmul(hr[:], nr_sb[:, r, :], grr[:],
                                     start=True, stop=False)
                    nc.tensor.matmul(hr[:], nn_sb[:, r, :], gri[:],
                                     start=False, stop=True)
                    nc.tensor.matmul(hi[:], ni_sb[:, r, :], grr[:],
                                     start=True, stop=False)
                    nc.tensor.matmul(hi[:], nr_sb[:, r, :], gri[:],
                                     start=False, stop=True)
                    prt = spool.tile([128, H], BF16, tag="prt")
                    pit = spool.tile([128, H], BF16, tag="pit")
                    nc.scalar.square(prt[:], hr[:])
                    nc.scalar.square(pit[:], hi[:])
                    t = spool.tile([128, H], BF16, tag="t")
                    nc.vector.tensor_add(t[:], prt[:], pit[:])
                    gs = spool.tile([128, H], BF16, tag="gs")
                    col = pr * 4 + r
                    nc.vector.scalar_tensor_tensor(
                        out=gs[:], in0=t[:], scalar=0.0, in1=wt_sb[:, r, :],
                        op0=ALU.bypass, op1=ALU.mult,
                        accum_out=acc[:, col: col + 1])

            def half(zr, zi, mA, mB, tagp):
                # stage-1 blocks mA, mB: o1_m = (Z^T D)[128m:128(m+1), :]
                # via 4M fp8 DoubleRow (c-chunk pairs), then L1 butterfly
                # s = o1_mA + o1_mB, d = o1_mA - o1_mB.
                par = ps1.tile([128, H], F32, tag="par")
                pai = ps1.tile([128, H], F32, tag="pai")
                pbr = ps1.tile([128, H], F32, tag="pbr")
                pbi = ps1.tile([128, H], F32, tag="pbi")
                for ps, m in ((par, mA), (pbr, mB)):
                    sl = slice(m * 128, (m + 1) * 128)
                    first = True
                    for cp in (0, 2):
                        cs = slice(cp, cp + 2)
                        nc.tensor.matmul(ps[:], zr[:, cs, sl], dr_sb[:, cs, :],
                                         start=first, stop=False,
                                         perf_mode=DRMODE)
                        nc.tensor.matmul(ps[:], zi[:, cs, sl], dn_sb[:, cs, :],
                                         start=False, stop=(cp == 2),
                                         perf_mode=DRMODE)
                        first = False
                for ps, m in ((pai, mA), (pbi, mB)):
                    sl = slice(m * 128, (m + 1) * 128)
                    first = True
                    for cp in (0, 2):
                        cs = slice(cp, cp + 2)
                        nc.tensor.matmul(ps[:], zr[:, cs, sl], di_sb[:, cs, :],
                                         start=first, stop=False,
                                         perf_mode=DRMODE)
                        nc.tensor.matmul(ps[:], zi[:, cs, sl], dr_sb[:, cs, :],
                                         start=False, stop=(cp == 2),
                                         perf_mode=DRMODE)
                        first = False
                car = bpool.tile([128, H], BF16, tag=f"c{tagp}r")
                cai = bpool.tile([128, H], BF16, tag=f"c{tagp}i")
                nc.scalar.copy(car[:], par[:])
                nc.scalar.copy(cai[:], pai[:])
                sr = bpool.tile([128, H], BF16, tag=f"s{tagp}r")
                si = bpool.tile([128, H], BF16, tag=f"s{tagp}i")
                dr_ = bpool.tile([128, H], BF16, tag=f"d{tagp}r")
                di_ = bpool.tile([128, H], BF16, tag=f"d{tagp}i")
                nc.vector.tensor_add(sr[:], car[:], pbr[:])
                nc.vector.tensor_sub(dr_[:], car[:], pbr[:])
                nc.vector.tensor_add(si[:], cai[:], pbi[:])
                nc.vector.tensor_sub(di_[:], cai[:], pbi[:])
                return sr, si, dr_, di_

            rep_ctx = (
                tc.For_i(0, repeat, 1,
                         hint_engines=(mybir.EngineType.PE,
                                       mybir.EngineType.DVE))
                if repeat is not None else nullcontext()
            )
            with rep_ctx:
              pending = None
              for pr in range(PAIRS):
                i1, i2 = 2 * pr, 2 * pr + 1
                zr = zpool.tile([128, 4, H], FP8, tag="zr")
                zi = zpool.tile([128, 4, H], FP8, tag="zi")
                for c in range(4):
                    p1c = loads.tile([128, H], F32, tag=f"p1t{c}")
                    g1c = loads.tile([128, H], F32, tag=f"g1t{c}")
                    p2c = loads.tile([128, H], F32, tag=f"p2t{c}")
                    g2c = loads.tile([128, H], F32, tag=f"g2t{c}")
                    nc.sync.dma_start(out=p1c[:], in_=r4(pred.ap()[i1])[:, c, :])
                    nc.sync.dma_start(out=g1c[:], in_=r4(gt.ap()[i1])[:, c, :])
                    nc.sync.dma_start(out=p2c[:], in_=r4(pred.ap()[i2])[:, c, :])
                    nc.sync.dma_start(out=g2c[:], in_=r4(gt.ap()[i2])[:, c, :])
                    if pr == 0:
                        # interleave const DMAs between data chunks
                        if c == 0:
                            nc.sync.dma_start(out=dr_sb[:], in_=r4(d_r.ap()))
                            nc.sync.dma_start(out=di_sb[:], in_=r4(d_i.ap()))
                        elif c == 1:
                            nc.sync.dma_start(out=dn_sb[:], in_=r4(d_n.ap()))
                            nc.sync.dma_start(out=nr_sb[:], in_=n_r.ap())
                        elif c == 2:
                            nc.sync.dma_start(out=ni_sb[:], in_=n_i.ap())
                            nc.sync.dma_start(out=nn_sb[:], in_=n_n.ap())
                        else:
                            nc.sync.dma_start(out=wt_sb[:], in_=wt.ap())
                    zre.tensor_sub(zr[:, c, :], p1c[:], g1c[:])
                    nc.vector.tensor_sub(zi[:, c, :], p2c[:], g2c[:])

                s02r, s02i, d02r, d02i = half(zr, zi, 0, 2, "a")
                if pending is not None:
                    stage2(*pending)
                s13r, s13i, d13r, d13i = half(zr, zi, 1, 3, "b")

                # L2 butterfly: g_r = sum_m (-i)^{mr} o1_m
                g = [gpool.tile([128, H], BF16, tag=f"g{k}", name=f"g{k}")
                     for k in range(8)]
                l2e.tensor_add(g[0][:], s02r[:], s13r[:])   # g0r
                l2e.tensor_add(g[1][:], s02i[:], s13i[:])   # g0i
                l2e.tensor_add(g[2][:], d02r[:], d13i[:])   # g1r
                l2e.tensor_sub(g[3][:], d02i[:], d13r[:])   # g1i
                l2e.tensor_sub(g[4][:], s02r[:], s13r[:])   # g2r
                l2e.tensor_sub(g[5][:], s02i[:], s13i[:])   # g2i
                l2e.tensor_sub(g[6][:], d02r[:], d13i[:])   # g3r
                l2e.tensor_add(g[7][:], d02i[:], d13r[:])   # g3i
                pending = (g, pr)
              stage2(*pending)

              nc.sync.dma_start(out=out.ap(), in_=acc[:])

    nc.compile()
    return nc


def _build_nc_v4(repeat=None, nwarm=16, subs="eng", ldchunks=2,
                 loop_warm=0, tail_chunk=True, t_eng="dve", sub_eng="mix", psb=1,
                 l1d_eng="dve"):
    """v4: DMA-accum subtraction (no engine sub ops), fp8-DR dense stage 1,
    stage 2 radix-4 DIF with BOTH butterfly levels folded:
      L1 (s/d of block pairs) on DVE from PSUM (4 batched ops/pair),
      L2 folded into 12 DoubleRow constant stacks (16 DR-MMs/pair).
    Per-pair engine work: PE 48 DR-MMs; DVE 4 L1 + 2 t + 2 stt;
    ACT 2 copies + 4 squares; Pool only SWDGE descriptor generation."""
    from contextlib import nullcontext
    nc = bacc.Bacc("TRN2", target_bir_lowering=False, debug=False,
                   num_devices=N_CORES)
    pred = nc.dram_tensor("pred", [IMGS_PER_CORE, H, H], F32, kind="ExternalInput")
    gt = nc.dram_tensor("gt", [IMGS_PER_CORE, H, H], F32, kind="ExternalInput")
    d_r = nc.dram_tensor("d_r", [H, H], FP8, kind="ExternalInput")
    d_i = nc.dram_tensor("d_i", [H, H], FP8, kind="ExternalInput")
    d_n = nc.dram_tensor("d_n", [H, H], FP8, kind="ExternalInput")
    st = nc.dram_tensor("st", [128, 12, 2, 128], FP8, kind="ExternalInput")
    wt = nc.dram_tensor("wt", [128, 4, H], BF16, kind="ExternalInput")
    out = nc.dram_tensor("out", [128, PAIRS * 2 + 2], F32, kind="ExternalOutput")

    def r4(ap):  # [512, 512] dram view -> [128 part, 4 chunks, 512]
        return ap.rearrange("(c p) w -> p c w", p=128)

    with tile.TileContext(nc) as tc:
        with (
            tc.tile_pool(name="consts", bufs=1) as consts,
            tc.tile_pool(name="loads", bufs=2) as loads,
            tc.tile_pool(name="zpool", bufs=2) as zpool,
            tc.tile_pool(name="sdpool", bufs=2) as sdpool,
            tc.tile_pool(name="cpool", bufs=2) as cpool,
            tc.tile_pool(name="spool", bufs=2) as spool,
            tc.tile_pool(name="ps1", bufs=1, space="PSUM") as ps1,
            tc.tile_pool(name="ps1b", bufs=(2 if psb == 2 else 1),
                         space="PSUM") as ps1b,
            tc.tile_pool(name="ps2", bufs=(1 if psb == 2 else 2),
                         space="PSUM") as ps2,
        ):
            dr_sb = consts.tile([128, 4, H], FP8)
            di_sb = consts.tile([128, 4, H], FP8)
            dn_sb = consts.tile([128, 4, H], FP8)
            st_sb = consts.tile([128, 12, 2, 128], FP8)
            wt_sb = consts.tile([128, 4, H], BF16)
            acc = consts.tile([128, PAIRS * 2 + 2], F32)
            warm = consts.tile([128, H], BF16)
            nc.vector.memset(warm[:], 0.0)
            if psb == 2:
                wps = ps2.tile([128, H], F32, tag="hr")
                wv = wps[:]
            else:
                wps = ps2.tile([128, 2, H], F32, tag="h2")
                wv = wps[:, 0, :]
            for i in range(nwarm):
                nc.tensor.matmul(wv, warm[:, 0:128], warm[:],
                                 start=(i == 0), stop=(i == nwarm - 1))

            def stage2rp(sd, pr, rp, chunked=False, t_eng="dve"):
                # H_r = N_r^T g_r with the L2 butterfly folded into the
                # stacks; per r: Hr = A^T pl_r + B^T pl_i ; Hi = C^T pl_r
                # + A^T pl_i (pl = (P0,P2) for r even, (P1,P3) for r odd).
                te = nc.gpsimd if t_eng == "pool" else nc.vector
                if True:
                    pq = spool.tile([128, 2, 2, H], BF16, tag="pq")
                    for rr in range(2):
                        r = rp * 2 + rr
                        sdsel = 0 if r in (0, 2) else 1
                        ka = (r * 3 + 0)
                        kb = (r * 3 + 1)
                        kc = (r * 3 + 2)
                        rpair = sd[:, sdsel, :, 0, :]
                        ipair = sd[:, sdsel, :, 1, :]
                        if psb == 2:
                            hr = ps2.tile([128, H], F32, tag="hr")
                            hi = ps2.tile([128, H], F32, tag="hi")
                            h0, h1 = hr[:], hi[:]
                        else:
                            h2 = ps2.tile([128, 2, H], F32, tag="h2")
                            h0, h1 = h2[:, 0, :], h2[:, 1, :]
                        nc.tensor.matmul(h0, st_sb[:, ka, :, :],
                                         rpair, start=True, stop=False,
                                         perf_mode=DRMODE)
                        nc.tensor.matmul(h0, st_sb[:, kb, :, :],
                                         ipair, start=False, stop=True,
                                         perf_mode=DRMODE)
                        nc.tensor.matmul(h1, st_sb[:, kc, :, :],
                                         rpair, start=True, stop=False,
                                         perf_mode=DRMODE)
                        nc.tensor.matmul(h1, st_sb[:, ka, :, :],
                                         ipair, start=False, stop=True,
                                         perf_mode=DRMODE)
                        if psb == 2:
                            nc.scalar.square(pq[:, rr, 0, :], h0)
                            nc.scalar.square(pq[:, rr, 1, :], h1)
                        else:
                            nc.scalar.square(pq[:, rr, :, :], h2[:])
                        if chunked:
                            # last-pair tail shrink: per-r weighted accum
                            t1 = spool.tile([128, H], BF16, tag="t1")
                            te.tensor_add(t1[:], pq[:, rr, 0, :],
                                          pq[:, rr, 1, :])
                            g1 = spool.tile([128, H], BF16, tag="g1")
                            col = pr * 2 + rp * 2 + rr
                            nc.vector.scalar_tensor_tensor(
                                out=g1[:], in0=t1[:], scalar=0.0,
                                in1=wt_sb[:, r, :],
                                op0=ALU.bypass, op1=ALU.mult,
                                accum_out=acc[:, col: col + 1])
                    if not chunked:
                        t = spool.tile([128, 2, H], BF16, tag="t")
                        te.tensor_add(t[:], pq[:, :, 0, :], pq[:, :, 1, :])
                        gs = spool.tile([128, 2, H], BF16, tag="gs")
                        col = pr * 2 + rp
                        nc.vector.scalar_tensor_tensor(
                            out=gs[:], in0=t[:], scalar=0.0,
                            in1=wt_sb[:, rp * 2: rp * 2 + 2, :],
                            op0=ALU.bypass, op1=ALU.mult,
                            accum_out=acc[:, col: col + 1])

            def half(zr, zi, mA, mB, sd, hf):
                # stage-1 blocks mA, mB (4M fp8 DoubleRow, c-chunk pairs),
                # then L1 butterfly into sd[:, {s,d}, hf, :, :].
                pa = ps1.tile([128, 2, H], F32, tag="pa")
                pb = ps1b.tile([128, 2, H], F32, tag="pb")
                for ps, m in ((pa, mA), (pb, mB)):
                    sl = slice(m * 128, (m + 1) * 128)
                    for comp, (dc1, dc2) in enumerate(((dr_sb, dn_sb),
                                                       (di_sb, dr_sb))):
                        first = True
                        for cp in (0, 2):
                            cs = slice(cp, cp + 2)
                            nc.tensor.matmul(ps[:, comp, :], zr[:, cs, sl],
                                             dc1[:, cs, :], start=first,
                                             stop=False, perf_mode=DRMODE)
                            nc.tensor.matmul(ps[:, comp, :], zi[:, cs, sl],
                                             dc2[:, cs, :], start=False,
                                             stop=(cp == 2), perf_mode=DRMODE)
                            first = False
                ca = cpool.tile([128, 2, H], BF16, tag="ca")
                if l1d_eng == "pool":
                    # ca2 = 2*o1_A (ACT); s = 0.5*ca2 + o1_B (DVE STT);
                    # d = o1_A - o1_B = ca2 - s on GPSIMD (SBUF-only).
                    # (sign of d is irrelevant: d-planes enter |H|^2
                    # through linear maps only)
                    nc.scalar.mul(ca[:], pa[:], 2.0)
                    sv = sd[:, 0, hf, :, :]
                    nc.vector.scalar_tensor_tensor(
                        out=sv, in0=ca[:], scalar=0.5, in1=pb[:],
                        op0=ALU.mult, op1=ALU.add)
                    nc.gpsimd.tensor_sub(sd[:, 1, hf, :, :], ca[:], sv)
                else:
                    nc.scalar.copy(ca[:], pa[:])
                    nc.vector.tensor_add(sd[:, 0, hf, :, :], ca[:], pb[:])
                    nc.vector.tensor_sub(sd[:, 1, hf, :, :], ca[:], pb[:])

            rep_ctx = (
                tc.For_i(0, repeat, 1,
                         hint_engines=(mybir.EngineType.PE,
                                       mybir.EngineType.DVE))
                if repeat is not None else nullcontext()
            )
            with rep_ctx:
              pending = None
              for pr in range(PAIRS):
                i1, i2 = 2 * pr, 2 * pr + 1
                zr = zpool.tile([128, 4, H], FP8, tag="zr")
                zi = zpool.tile([128, 4, H], FP8, tag="zi")
                if subs == "dma":
                    # z = pred + (-gt) via DMA: cast-load -gt, then
                    # accum-add pred (SWDGE: out = in + out), per c-half to
                    # align with the DoubleRow c-pair consumption.
                    for ch in (0, 2):
                        cs = slice(ch, ch + 2)
                        nc.gpsimd.dma_start(out=zr[:, cs, :],
                                            in_=r4(gt.ap()[i1])[:, cs, :])
                        nc.gpsimd.dma_start(out=zr[:, cs, :],
                                            in_=r4(pred.ap()[i1])[:, cs, :],
                                            accum_op=ALU.add)
                        nc.gpsimd.dma_start(out=zi[:, cs, :],
                                            in_=r4(gt.ap()[i2])[:, cs, :])
                        nc.gpsimd.dma_start(out=zi[:, cs, :],
                                            in_=r4(pred.ap()[i2])[:, cs, :],
                                            accum_op=ALU.add)
                else:
                    # HWDGE chunked loads + engine adds (gt pre-negated on
                    # host): zr on GPSIMD, zi on DVE, per c-half.
                    for ch in (0, 2):
                        cs = slice(ch, ch + 2)
                        p1h = loads.tile([128, 2, H], F32, tag=f"p1h{ch}")
                        g1h = loads.tile([128, 2, H], F32, tag=f"g1h{ch}")
                        p2h = loads.tile([128, 2, H], F32, tag=f"p2h{ch}")
                        g2h = loads.tile([128, 2, H], F32, tag=f"g2h{ch}")
                        nc.sync.dma_start(out=p1h[:], in_=r4(pred.ap()[i1])[:, cs, :])
                        nc.sync.dma_start(out=g1h[:], in_=r4(gt.ap()[i1])[:, cs, :])
                        nc.sync.dma_start(out=p2h[:], in_=r4(pred.ap()[i2])[:, cs, :])
                        nc.sync.dma_start(out=g2h[:], in_=r4(gt.ap()[i2])[:, cs, :])
                        nc.gpsimd.tensor_add(zr[:, cs, :], p1h[:], g1h[:])
                        ze = nc.gpsimd if sub_eng == "pool" else nc.vector
                        ze.tensor_add(zi[:, cs, :], p2h[:], g2h[:])
                if pr == 0:
                    nc.sync.dma_start(out=dr_sb[:], in_=r4(d_r.ap()))
                    nc.sync.dma_start(out=di_sb[:], in_=r4(d_i.ap()))
                    nc.sync.dma_start(out=dn_sb[:], in_=r4(d_n.ap()))
                    nc.sync.dma_start(out=st_sb[:], in_=st.ap())
                    nc.sync.dma_start(out=wt_sb[:], in_=wt.ap())

                if pr == 0 and loop_warm:
                    # keep the HAM clock-gate warm through the load head
                    wps2 = ps1.tile([128, 2, H], F32, tag="pa")
                    for i in range(loop_warm):
                        nc.tensor.matmul(wps2[:, 0, :], warm[:, 0:128],
                                         warm[:], start=(i == 0),
                                         stop=(i == loop_warm - 1))
                sd = sdpool.tile([128, 2, 2, 2, H], FP8, tag="sd")
                half(zr, zi, 0, 2, sd, 0)
                if pending is not None:
                    stage2rp(*pending, 0, t_eng=t_eng)
                half(zr, zi, 1, 3, sd, 1)
                if pending is not None:
                    stage2rp(*pending, 1, t_eng=t_eng)
                pending = (sd, pr)
              stage2rp(*pending, 0, chunked=tail_chunk, t_eng=t_eng)
              stage2rp(*pending, 1, chunked=tail_chunk, t_eng=t_eng)

              nc.sync.dma_start(out=out.ap(), in_=acc[:])

    nc.compile()
    return nc


BUILD = _build_nc_v4


def _host_constants():
    """Precompute the device constant tensors (shared across cores)."""
    e4 = ml_dtypes.float8_e4m3
    rng = np.random.default_rng(DITHER_SEED)
    su = np.exp(rng.uniform(0.0, np.log(2.0), H))        # stage-1 col dither
    sq = np.exp(rng.uniform(0.0, np.log(2.0), 128))      # stage-2 col dither
    j = np.arange(H, dtype=np.float64)
    ang = 2.0 * np.pi * np.outer(j, j) / H
    scale = 1.0 / np.sqrt(H)
    Dre = np.cos(ang) * scale
    Dim = -np.sin(ang) * scale
    drb = np.ascontiguousarray((Dre * S1 * su[None, :]).astype(np.float32).astype(e4))
    dib = np.ascontiguousarray((Dim * S1 * su[None, :]).astype(np.float32).astype(e4))
    dnb = np.ascontiguousarray((-Dim * S1 * su[None, :]).astype(np.float32).astype(e4))
    # stage-2 stacks: quantize N_r first, then build stacks from the
    # quantized planes (so A/B/C/D share bit-identical entries).
    b = np.arange(128, dtype=np.float64)
    y = np.arange(128, dtype=np.float64)
    base = 2.0 * np.pi * np.outer(b, y) / 128.0
    Nq = []
    for r in range(4):
        a = base + (2.0 * np.pi * b * r / H)[:, None]
        m_r = np.cos(a) * scale * S2 * sq[None, :]
        m_i = -np.sin(a) * scale * S2 * sq[None, :]
        Nq.append((m_r.astype(np.float32).astype(e4).astype(np.float32),
                   m_i.astype(np.float32).astype(e4).astype(np.float32)))
    sth = np.empty((128, 12, 2, 128), np.float32)
    for r, (Nr, Ni) in enumerate(Nq):
        if r == 0:
            stacks = [(Nr, Nr), (-Ni, -Ni), (Ni, Ni)]
        elif r == 1:
            stacks = [(Nr, Ni), (-Ni, Nr), (Ni, -Nr)]
        elif r == 2:
            stacks = [(Nr, -Nr), (-Ni, Ni), (Ni, -Ni)]
        else:
            stacks = [(Nr, -Ni), (-Ni, -Nr), (Ni, Nr)]
        for kk, (m0, m1) in enumerate(stacks):
            sth[:, r * 3 + kk, 0, :] = m0
            sth[:, r * 3 + kk, 1, :] = m1
    stb = np.ascontiguousarray(sth.astype(e4))
    return su, sq, drb, dib, dnb, stb


def kernel(predictions, ground_truths, band_weights, band_masks):
    global last_results, last_nc, last_in_maps
    pred = np.ascontiguousarray(np.asarray(predictions, dtype=np.float32))
    gt = np.ascontiguousarray(np.asarray(ground_truths, dtype=np.float32))
    bw = np.asarray(band_weights, dtype=np.float64)
    bm = np.asarray(band_masks, dtype=np.float64)

    wmap = np.einsum('b,bhw->hw', bw, bm)          # shifted coords
    wu = np.fft.ifftshift(wmap)                     # [u_h, v_w] unshifted
    su, sq, drb, dib, dnb, stb = _host_constants()
    # weight tile Wt[y, r, u] = Wu[u, 4y+r] / (su[u]^2 sq[y]^2 S1^2 S2^2)
    yv = np.arange(128)
    wth = np.empty((128, 4, H), np.float64)
    for r in range(4):
        wth[:, r, :] = wu[:, 4 * yv + r].T
    wth /= (su[None, None, :] ** 2) * (sq[:, None, None] ** 2) \
        * (S1 * S1 * S2 * S2)
    bf = ml_dtypes.bfloat16
    wtb = np.ascontiguousarray(wth.astype(np.float32).astype(bf))

    pred_r = pred.reshape(N_CORES, IMGS_PER_CORE, H, H)
    gt_r = gt.reshape(N_CORES, IMGS_PER_CORE, H, H)
    in_maps = [
        {
            "pred": np.ascontiguousarray(pred_r[c]),
            "gt": np.ascontiguousarray(-gt_r[c]),
            "d_r": drb, "d_i": dib, "d_n": dnb,
            "st": stb, "wt": wtb,
        }
        for c in range(N_CORES)
    ]

    nc = BUILD()
    last_nc, last_in_maps = nc, in_maps
    res = run_bass_kernel_spmd(nc, in_maps, core_ids=list(range(N_CORES)))
    last_results = res
    total = np.float64(0.0)
    for r in res.results:
        total += r["out"].astype(np.float64).sum()
    loss = total / float(N * C * H * H)
    return np.float32(loss)


# revision 28
# speedup vs baseline: 1.2069x; 1.0075x over previous
"""Trainium2 Bass kernel for DifferentiableWeightedRadialFrequencyLoss.

Math:
  loss = sum_{n,c,u,v} Wmap[u,v] * |FFT2(pred-gt)[u,v]|^2 / size
with Wmap = sum_b w_b * mask_b (bands disjoint), in unshifted (ifftshift)
frequency coordinates.

Device algorithm (per core, 12 images = 6 pairs), v3:
  - pack two real images per complex FFT: Z = E1 + i*E2 (Wmap is symmetric
    under (u,v) -> (-u,-v), so cross terms cancel exactly).
  - stage 1 (h-transform): dense o1 = Z^T @ D as 4M complex products that
    accumulate in PSUM (no DVE combines), with fp8e4 operands in DoubleRow
    perf mode (K=256 per matmul: c-chunk pairs in the free axis).  D carries
    a fixed random per-column dither scale (compensated in the weight map)
    to decorrelate fp8 quantization error, plus a global x8 scale.
  - stage 2 (w-transform): radix-4 DIF.  The four natural 128-row blocks of
    o1 are butterflied (L1 on DVE from PSUM, L2 on GPSIMD in SBUF) into
    g_r = sum_m (-i)^{mr} o1_m, then four independent 128-point transforms
    with twiddle-folded bf16 constants N_r[b,y] = w128^{by} w512^{br}/sqrt(512)
    run as 4M complex matmuls (K=128).  Output F[4y+r, u] = (N_r^T g_r)[y,u].
  - power: ACT squares PSUM H-planes, DVE adds + weighted row-sum
    (scalar_tensor_tensor accum_out) against the host-precomputed,
    dither-compensated weight map Wt[y,r,u] = Wu[u,4y+r]/(su[u]^2 s1^2).
Host: shard batch across 8 cores, sum partial accumulators, divide by size.
"""

import numpy as np
import ml_dtypes

import concourse.bass as bass
import concourse.bacc as bacc
import concourse.tile as tile
from concourse import mybir
from concourse.bass_utils import run_bass_kernel_spmd

N_CORES = 8
N, C, H = 32, 3, 512
NUM_BANDS = 16
IMGS_PER_CORE = (N // N_CORES) * C          # 12
PAIRS = IMGS_PER_CORE // 2                  # 6
F32 = mybir.dt.float32
BF16 = mybir.dt.bfloat16
FP8 = mybir.dt.float8e4
ALU = mybir.AluOpType
DRMODE = mybir.MatmulPerfMode.DoubleRow

S1 = 4.0            # global scale on D (folded out via wt)
S2 = 8.0            # global scale on the stage-2 stacks (folded out via wt)
DITHER_SEED = 7     # per-column dither on D / stacks (folded out via wt)

# exposed for test.py introspection
last_results = None
last_nc = None
last_in_maps = None


def _build_nc_v3(repeat=None, nwarm=16, zr_eng="pool", l2_eng="pool",
                 ldbufs=2, zbufs=2):
    """v3: fp8-DoubleRow dense stage 1 + radix-4 DIF bf16 stage 2.

    repeat=N wraps the whole per-core body in a hardware For_i loop for
    steady-state timing (same contract as the baseline builds)."""
    from contextlib import nullcontext
    nc = bacc.Bacc("TRN2", target_bir_lowering=False, debug=False,
                   num_devices=N_CORES)
    pred = nc.dram_tensor("pred", [IMGS_PER_CORE, H, H], F32, kind="ExternalInput")
    gt = nc.dram_tensor("gt", [IMGS_PER_CORE, H, H], F32, kind="ExternalInput")
    d_r = nc.dram_tensor("d_r", [H, H], FP8, kind="ExternalInput")   # Re(D)*s
    d_i = nc.dram_tensor("d_i", [H, H], FP8, kind="ExternalInput")   # Im(D)*s
    d_n = nc.dram_tensor("d_n", [H, H], FP8, kind="ExternalInput")   # -Im(D)*s
    n_r = nc.dram_tensor("n_r", [128, 4, 128], BF16, kind="ExternalInput")
    n_i = nc.dram_tensor("n_i", [128, 4, 128], BF16, kind="ExternalInput")
    n_n = nc.dram_tensor("n_n", [128, 4, 128], BF16, kind="ExternalInput")
    wt = nc.dram_tensor("wt", [128, 4, H], BF16, kind="ExternalInput")
    out = nc.dram_tensor("out", [128, PAIRS * 4], F32, kind="ExternalOutput")

    def r4(ap):  # [512, 512] dram view -> [128 part, 4 chunks, 512]
        return ap.rearrange("(c p) w -> p c w", p=128)

    with tile.TileContext(nc) as tc:
        with (
            tc.tile_pool(name="consts", bufs=1) as consts,
            tc.tile_pool(name="loads", bufs=ldbufs) as loads,
            tc.tile_pool(name="zpool", bufs=zbufs) as zpool,
            tc.tile_pool(name="bpool", bufs=2) as bpool,
            tc.tile_pool(name="gpool", bufs=2) as gpool,
            tc.tile_pool(name="spool", bufs=2) as spool,
            tc.tile_pool(name="ps1", bufs=1, space="PSUM") as ps1,
            tc.tile_pool(name="ps2", bufs=2, space="PSUM") as ps2,
        ):
            dr_sb = consts.tile([128, 4, H], FP8)
            di_sb = consts.tile([128, 4, H], FP8)
            dn_sb = consts.tile([128, 4, H], FP8)
            nr_sb = consts.tile([128, 4, 128], BF16)
            ni_sb = consts.tile([128, 4, 128], BF16)
            nn_sb = consts.tile([128, 4, 128], BF16)
            wt_sb = consts.tile([128, 4, H], BF16)
            acc = consts.tile([128, PAIRS * 4], F32)
            # warmup: keep PE busy during the DMA lead-in so the HAM
            # clock-gate is at full rate when real matmuls start.
            warm = consts.tile([128, H], BF16)
            nc.vector.memset(warm[:], 0.0)
            wps = ps2.tile([128, H], F32, tag="hr")
            for i in range(nwarm):
                nc.tensor.matmul(wps[:], warm[:, 0:128], warm[:],
                                 start=(i == 0), stop=(i == nwarm - 1))

            zre = nc.gpsimd if zr_eng == "pool" else nc.vector
            l2e = nc.gpsimd if l2_eng == "pool" else nc.vector

            def stage2(g, pr):
                # 4 independent 128-point transforms + weighted power
                for r in range(4):
                    grr, gri = g[2 * r], g[2 * r + 1]
                    hr = ps2.tile([128, H], F32, tag="hr")
                    hi = ps2.tile([128, H], F32, tag="hi")
                    nc.tensor.matmul(hr[:], nr_sb[:, r, :], grr[:],
                                     start=True, stop=False)
                    nc.tensor.matmul(hr[:], nn_sb[:, r, :], gri[:],
                                     start=False, stop=True)
                    nc.tensor.matmul(hi[:], ni_sb[:, r, :], grr[:],
                                     start=True, stop=False)
                    nc.tensor.matmul(hi[:], nr_sb[:, r, :], gri[:],
                                     start=False, stop=True)
                    prt = spool.tile([128, H], BF16, tag="prt")
                    pit = spool.tile([128, H], BF16, tag="pit")
                    nc.scalar.square(prt[:], hr[:])
                    nc.scalar.square(pit[:], hi[:])
                    t = spool.tile([128, H], BF16, tag="t")
                    nc.vector.tensor_add(t[:], prt[:], pit[:])
                    gs = spool.tile([128, H], BF16, tag="gs")
                    col = pr * 4 + r
                    nc.vector.scalar_tensor_tensor(
                        out=gs[:], in0=t[:], scalar=0.0, in1=wt_sb[:, r, :],
                        op0=ALU.bypass, op1=ALU.mult,
                        accum_out=acc[:, col: col + 1])

            def half(zr, zi, mA, mB, tagp):
                # stage-1 blocks mA, mB: o1_m = (Z^T D)[128m:128(m+1), :]
                # via 4M fp8 DoubleRow (c-chunk pairs), then L1 butterfly
                # s = o1_mA + o1_mB, d = o1_mA - o1_mB.
                par = ps1.tile([128, H], F32, tag="par")
                pai = ps1.tile([128, H], F32, tag="pai")
                pbr = ps1.tile([128, H], F32, tag="pbr")
                pbi = ps1.tile([128, H], F32, tag="pbi")
                for ps, m in ((par, mA), (pbr, mB)):
                    sl = slice(m * 128, (m + 1) * 128)
                    first = True
                    for cp in (0, 2):
                        cs = slice(cp, cp + 2)
                        nc.tensor.matmul(ps[:], zr[:, cs, sl], dr_sb[:, cs, :],
                                         start=first, stop=False,
                                         perf_mode=DRMODE)
                        nc.tensor.matmul(ps[:], zi[:, cs, sl], dn_sb[:, cs, :],
                                         start=False, stop=(cp == 2),
                                         perf_mode=DRMODE)
                        first = False
                for ps, m in ((pai, mA), (pbi, mB)):
                    sl = slice(m * 128, (m + 1) * 128)
                    first = True
                    for cp in (0, 2):
                        cs = slice(cp, cp + 2)
                        nc.tensor.matmul(ps[:], zr[:, cs, sl], di_sb[:, cs, :],
                                         start=first, stop=False,
                                         perf_mode=DRMODE)
                        nc.tensor.matmul(ps[:], zi[:, cs, sl], dr_sb[:, cs, :],
                                         start=False, stop=(cp == 2),
                                         perf_mode=DRMODE)
                        first = False
                car = bpool.tile([128, H], BF16, tag=f"c{tagp}r")
                cai = bpool.tile([128, H], BF16, tag=f"c{tagp}i")
                nc.scalar.copy(car[:], par[:])
                nc.scalar.copy(cai[:], pai[:])
                sr = bpool.tile([128, H], BF16, tag=f"s{tagp}r")
                si = bpool.tile([128, H], BF16, tag=f"s{tagp}i")
                dr_ = bpool.tile([128, H], BF16, tag=f"d{tagp}r")
                di_ = bpool.tile([128, H], BF16, tag=f"d{tagp}i")
                nc.vector.tensor_add(sr[:], car[:], pbr[:])
                nc.vector.tensor_sub(dr_[:], car[:], pbr[:])
                nc.vector.tensor_add(si[:], cai[:], pbi[:])
                nc.vector.tensor_sub(di_[:], cai[:], pbi[:])
                return sr, si, dr_, di_

            rep_ctx = (
                tc.For_i(0, repeat, 1,
                         hint_engines=(mybir.EngineType.PE,
                                       mybir.EngineType.DVE))
                if repeat is not None else nullcontext()
            )
            with rep_ctx:
              pending = None
              for pr in range(PAIRS):
                i1, i2 = 2 * pr, 2 * pr + 1
                zr = zpool.tile([128, 4, H], FP8, tag="zr")
                zi = zpool.tile([128, 4, H], FP8, tag="zi")
                for c in range(4):
                    p1c = loads.tile([128, H], F32, tag=f"p1t{c}")
                    g1c = loads.tile([128, H], F32, tag=f"g1t{c}")
                    p2c = loads.tile([128, H], F32, tag=f"p2t{c}")
                    g2c = loads.tile([128, H], F32, tag=f"g2t{c}")
                    nc.sync.dma_start(out=p1c[:], in_=r4(pred.ap()[i1])[:, c, :])
                    nc.sync.dma_start(out=g1c[:], in_=r4(gt.ap()[i1])[:, c, :])
                    nc.sync.dma_start(out=p2c[:], in_=r4(pred.ap()[i2])[:, c, :])
                    nc.sync.dma_start(out=g2c[:], in_=r4(gt.ap()[i2])[:, c, :])
                    if pr == 0:
                        # interleave const DMAs between data chunks
                        if c == 0:
                            nc.sync.dma_start(out=dr_sb[:], in_=r4(d_r.ap()))
                            nc.sync.dma_start(out=di_sb[:], in_=r4(d_i.ap()))
                        elif c == 1:
                            nc.sync.dma_start(out=dn_sb[:], in_=r4(d_n.ap()))
                            nc.sync.dma_start(out=nr_sb[:], in_=n_r.ap())
                        elif c == 2:
                            nc.sync.dma_start(out=ni_sb[:], in_=n_i.ap())
                            nc.sync.dma_start(out=nn_sb[:], in_=n_n.ap())
                        else:
                            nc.sync.dma_start(out=wt_sb[:], in_=wt.ap())
                    zre.tensor_sub(zr[:, c, :], p1c[:], g1c[:])
                    nc.vector.tensor_sub(zi[:, c, :], p2c[:], g2c[:])

                s02r, s02i, d02r, d02i = half(zr, zi, 0, 2, "a")
                if pending is not None:
                    stage2(*pending)
                s13r, s13i, d13r, d13i = half(zr, zi, 1, 3, "b")

                # L2 butterfly: g_r = sum_m (-i)^{mr} o1_m
                g = [gpool.tile([128, H], BF16, tag=f"g{k}", name=f"g{k}")
                     for k in range(8)]
                l2e.tensor_add(g[0][:], s02r[:], s13r[:])   # g0r
                l2e.tensor_add(g[1][:], s02i[:], s13i[:])   # g0i
                l2e.tensor_add(g[2][:], d02r[:], d13i[:])   # g1r
                l2e.tensor_sub(g[3][:], d02i[:], d13r[:])   # g1i
                l2e.tensor_sub(g[4][:], s02r[:], s13r[:])   # g2r
                l2e.tensor_sub(g[5][:], s02i[:], s13i[:])   # g2i
                l2e.tensor_sub(g[6][:], d02r[:], d13i[:])   # g3r
                l2e.tensor_add(g[7][:], d02i[:], d13r[:])   # g3i
                pending = (g, pr)
              stage2(*pending)

              nc.sync.dma_start(out=out.ap(), in_=acc[:])

    nc.compile()
    return nc


def _build_nc_v4(repeat=None, nwarm=16, subs="eng", ldchunks=2,
                 loop_warm=0, tail_chunk=True, t_eng="dve", sub_eng="mix", psb=1,
                 l1d_eng="dve"):
    """v4: DMA-accum subtraction (no engine sub ops), fp8-DR dense stage 1,
    stage 2 radix-4 DIF with BOTH butterfly levels folded:
      L1 (s/d of block pairs) on DVE from PSUM (4 batched ops/pair),
      L2 folded into 12 DoubleRow constant stacks (16 DR-MMs/pair).
    Per-pair engine work: PE 48 DR-MMs; DVE 4 L1 + 2 t + 2 stt;
    ACT 2 copies + 4 squares; Pool only SWDGE descriptor generation."""
    from contextlib import nullcontext
    nc = bacc.Bacc("TRN2", target_bir_lowering=False, debug=False,
                   num_devices=N_CORES)
    pred = nc.dram_tensor("pred", [IMGS_PER_CORE, H, H], F32, kind="ExternalInput")
    gt = nc.dram_tensor("gt", [IMGS_PER_CORE, H, H], F32, kind="ExternalInput")
    d_r = nc.dram_tensor("d_r", [H, H], FP8, kind="ExternalInput")
    d_i = nc.dram_tensor("d_i", [H, H], FP8, kind="ExternalInput")
    d_n = nc.dram_tensor("d_n", [H, H], FP8, kind="ExternalInput")
    st = nc.dram_tensor("st", [128, 12, 2, 128], FP8, kind="ExternalInput")
    wt = nc.dram_tensor("wt", [128, 4, H], BF16, kind="ExternalInput")
    out = nc.dram_tensor("out", [128, PAIRS * 2 + 2], F32, kind="ExternalOutput")

    def r4(ap):  # [512, 512] dram view -> [128 part, 4 chunks, 512]
        return ap.rearrange("(c p) w -> p c w", p=128)

    with tile.TileContext(nc) as tc:
        with (
            tc.tile_pool(name="consts", bufs=1) as consts,
            tc.tile_pool(name="loads", bufs=2) as loads,
            tc.tile_pool(name="zpool", bufs=2) as zpool,
            tc.tile_pool(name="sdpool", bufs=2) as sdpool,
            tc.tile_pool(name="cpool", bufs=2) as cpool,
            tc.tile_pool(name="spool", bufs=2) as spool,
            tc.tile_pool(name="ps1", bufs=1, space="PSUM") as ps1,
            tc.tile_pool(name="ps1b", bufs=(2 if psb == 2 else 1),
                         space="PSUM") as ps1b,
            tc.tile_pool(name="ps2", bufs=(1 if psb == 2 else 2),
                         space="PSUM") as ps2,
        ):
            dr_sb = consts.tile([128, 4, H], FP8)
            di_sb = consts.tile([128, 4, H], FP8)
            dn_sb = consts.tile([128, 4, H], FP8)
            st_sb = consts.tile([128, 12, 2, 128], FP8)
            wt_sb = consts.tile([128, 4, H], BF16)
            acc = consts.tile([128, PAIRS * 2 + 2], F32)
            warm = consts.tile([128, H], BF16)
            nc.vector.memset(warm[:], 0.0)
            if psb == 2:
                wps = ps2.tile([128, H], F32, tag="hr")
                wv = wps[:]
            else:
                wps = ps2.tile([128, 2, H], F32, tag="h2")
                wv = wps[:, 0, :]
            for i in range(nwarm):
                nc.tensor.matmul(wv, warm[:, 0:128], warm[:],
                                 start=(i == 0), stop=(i == nwarm - 1))

            def s2mm(sd, pr, rp):
                # stage-2 matmuls for r-pair rp; squares/reduce deferred to
                # s2post so ACT's FIFO serves the ca copies first.
                hs = []
                for rr in range(2):
                    r = rp * 2 + rr
                    sdsel = 0 if r in (0, 2) else 1
                    ka = (r * 3 + 0)
                    kb = (r * 3 + 1)
                    kc = (r * 3 + 2)
                    rpair = sd[:, sdsel, :, 0, :]
                    ipair = sd[:, sdsel, :, 1, :]
                    h2 = ps2.tile([128, 2, H], F32, tag="h2")
                    h0, h1 = h2[:, 0, :], h2[:, 1, :]
                    nc.tensor.matmul(h0, st_sb[:, ka, :, :],
                                     rpair, start=True, stop=False,
                                     perf_mode=DRMODE)
                    nc.tensor.matmul(h0, st_sb[:, kb, :, :],
                                     ipair, start=False, stop=True,
                                     perf_mode=DRMODE)
                    nc.tensor.matmul(h1, st_sb[:, kc, :, :],
                                     rpair, start=True, stop=False,
                                     perf_mode=DRMODE)
                    nc.tensor.matmul(h1, st_sb[:, ka, :, :],
                                     ipair, start=False, stop=True,
                                     perf_mode=DRMODE)
                    hs.append(h2)
                return hs

            def s2post(hs, pr, rp, chunked=False):
                pq = spool.tile([128, 2, 2, H], BF16, tag="pq")
                for rr in range(2):
                    r = rp * 2 + rr
                    nc.scalar.square(pq[:, rr, :, :], hs[rr][:])
                    if chunked:
                        t1 = spool.tile([128, H], BF16, tag="t1")
                        nc.vector.tensor_add(t1[:], pq[:, rr, 0, :],
                                             pq[:, rr, 1, :])
                        g1 = spool.tile([128, H], BF16, tag="g1")
                        col = pr * 2 + rp * 2 + rr
                        nc.vector.scalar_tensor_tensor(
                            out=g1[:], in0=t1[:], scalar=0.0,
                            in1=wt_sb[:, r, :],
                            op0=ALU.bypass, op1=ALU.mult,
                            accum_out=acc[:, col: col + 1])
                if not chunked:
                    t = spool.tile([128, 2, H], BF16, tag="t")
                    nc.vector.tensor_add(t[:], pq[:, :, 0, :], pq[:, :, 1, :])
                    gs = spool.tile([128, 2, H], BF16, tag="gs")
                    col = pr * 2 + rp
                    nc.vector.scalar_tensor_tensor(
                        out=gs[:], in0=t[:], scalar=0.0,
                        in1=wt_sb[:, rp * 2: rp * 2 + 2, :],
                        op0=ALU.bypass, op1=ALU.mult,
                        accum_out=acc[:, col: col + 1])

            def half(zr, zi, mA, mB, sd, hf):
                # stage-1 blocks mA, mB (4M fp8 DoubleRow, c-chunk pairs),
                # then L1 butterfly into sd[:, {s,d}, hf, :, :].
                pa = ps1.tile([128, 2, H], F32, tag="pa")
                pb = ps1b.tile([128, 2, H], F32, tag="pb")
                for ps, m in ((pa, mA), (pb, mB)):
                    sl = slice(m * 128, (m + 1) * 128)
                    for comp, (dc1, dc2) in enumerate(((dr_sb, dn_sb),
                                                       (di_sb, dr_sb))):
                        first = True
                        for cp in (0, 2):
                            cs = slice(cp, cp + 2)
                            nc.tensor.matmul(ps[:, comp, :], zr[:, cs, sl],
                                             dc1[:, cs, :], start=first,
                                             stop=False, perf_mode=DRMODE)
                            nc.tensor.matmul(ps[:, comp, :], zi[:, cs, sl],
                                             dc2[:, cs, :], start=False,
                                             stop=(cp == 2), perf_mode=DRMODE)
                            first = False
                ca = cpool.tile([128, 2, H], BF16, tag="ca")
                if l1d_eng == "pool":
                    # ca2 = 2*o1_A (ACT); s = 0.5*ca2 + o1_B (DVE STT);
                    # d = o1_A - o1_B = ca2 - s on GPSIMD (SBUF-only).
                    # (sign of d is irrelevant: d-planes enter |H|^2
                    # through linear maps only)
                    nc.scalar.mul(ca[:], pa[:], 2.0)
                    sv = sd[:, 0, hf, :, :]
                    nc.vector.scalar_tensor_tensor(
                        out=sv, in0=ca[:], scalar=0.5, in1=pb[:],
                        op0=ALU.mult, op1=ALU.add)
                    nc.gpsimd.tensor_sub(sd[:, 1, hf, :, :], ca[:], sv)
                else:
                    nc.scalar.copy(ca[:], pa[:])
                    nc.vector.tensor_add(sd[:, 0, hf, :, :], ca[:], pb[:])
                    nc.vector.tensor_sub(sd[:, 1, hf, :, :], ca[:], pb[:])

            rep_ctx = (
                tc.For_i(0, repeat, 1,
                         hint_engines=(mybir.EngineType.PE,
                                       mybir.EngineType.DVE))
                if repeat is not None else nullcontext()
            )
            with rep_ctx:
              pending = None
              for pr in range(PAIRS):
                i1, i2 = 2 * pr, 2 * pr + 1
                zr = zpool.tile([128, 4, H], FP8, tag="zr")
                zi = zpool.tile([128, 4, H], FP8, tag="zi")
                if subs == "dma":
                    # z = pred + (-gt) via DMA: cast-load -gt, then
                    # accum-add pred (SWDGE: out = in + out), per c-half to
                    # align with the DoubleRow c-pair consumption.
                    for ch in (0, 2):
                        cs = slice(ch, ch + 2)
                        nc.gpsimd.dma_start(out=zr[:, cs, :],
                                            in_=r4(gt.ap()[i1])[:, cs, :])
                        nc.gpsimd.dma_start(out=zr[:, cs, :],
                                            in_=r4(pred.ap()[i1])[:, cs, :],
                                            accum_op=ALU.add)
                        nc.gpsimd.dma_start(out=zi[:, cs, :],
                                            in_=r4(gt.ap()[i2])[:, cs, :])
                        nc.gpsimd.dma_start(out=zi[:, cs, :],
                                            in_=r4(pred.ap()[i2])[:, cs, :],
                                            accum_op=ALU.add)
                else:
                    # HWDGE chunked loads + engine adds (gt pre-negated on
                    # host): zr on GPSIMD, zi on DVE, per c-half.
                    for ch in (0, 2):
                        cs = slice(ch, ch + 2)
                        p1h = loads.tile([128, 2, H], F32, tag=f"p1h{ch}")
                        g1h = loads.tile([128, 2, H], F32, tag=f"g1h{ch}")
                        p2h = loads.tile([128, 2, H], F32, tag=f"p2h{ch}")
                        g2h = loads.tile([128, 2, H], F32, tag=f"g2h{ch}")
                        nc.sync.dma_start(out=p1h[:], in_=r4(pred.ap()[i1])[:, cs, :])
                        nc.sync.dma_start(out=g1h[:], in_=r4(gt.ap()[i1])[:, cs, :])
                        nc.sync.dma_start(out=p2h[:], in_=r4(pred.ap()[i2])[:, cs, :])
                        nc.sync.dma_start(out=g2h[:], in_=r4(gt.ap()[i2])[:, cs, :])
                        nc.gpsimd.tensor_add(zr[:, cs, :], p1h[:], g1h[:])
                        ze = nc.gpsimd if sub_eng == "pool" else nc.vector
                        ze.tensor_add(zi[:, cs, :], p2h[:], g2h[:])
                if pr == 0:
                    nc.sync.dma_start(out=dr_sb[:], in_=r4(d_r.ap()))
                    nc.sync.dma_start(out=di_sb[:], in_=r4(d_i.ap()))
                    nc.sync.dma_start(out=dn_sb[:], in_=r4(d_n.ap()))
                    nc.sync.dma_start(out=st_sb[:], in_=st.ap())
                    nc.sync.dma_start(out=wt_sb[:], in_=wt.ap())

                if pr == 0 and loop_warm:
                    # keep the HAM clock-gate warm through the load head
                    wps2 = ps1.tile([128, 2, H], F32, tag="pa")
                    for i in range(loop_warm):
                        nc.tensor.matmul(wps2[:, 0, :], warm[:, 0:128],
                                         warm[:], start=(i == 0),
                                         stop=(i == loop_warm - 1))
                sd = sdpool.tile([128, 2, 2, 2, H], FP8, tag="sd")
                half(zr, zi, 0, 2, sd, 0)
                if pending is not None:
                    hs0 = s2mm(pending[0], pending[1], 0)
                half(zr, zi, 1, 3, sd, 1)
                if pending is not None:
                    s2post(hs0, pending[1], 0)
                    hs1 = s2mm(pending[0], pending[1], 1)
                    s2post(hs1, pending[1], 1)
                pending = (sd, pr)
              hs0 = s2mm(pending[0], pending[1], 0)
              s2post(hs0, pending[1], 0, chunked=tail_chunk)
              hs1 = s2mm(pending[0], pending[1], 1)
              s2post(hs1, pending[1], 1, chunked=tail_chunk)

              nc.sync.dma_start(out=out.ap(), in_=acc[:])

    nc.compile()
    return nc


BUILD = _build_nc_v4


def _host_constants():
    """Precompute the device constant tensors (shared across cores)."""
    e4 = ml_dtypes.float8_e4m3
    rng = np.random.default_rng(DITHER_SEED)
    su = np.exp(rng.uniform(0.0, np.log(2.0), H))        # stage-1 col dither
    sq = np.exp(rng.uniform(0.0, np.log(2.0), 128))      # stage-2 col dither
    j = np.arange(H, dtype=np.float64)
    ang = 2.0 * np.pi * np.outer(j, j) / H
    scale = 1.0 / np.sqrt(H)
    Dre = np.cos(ang) * scale
    Dim = -np.sin(ang) * scale
    drb = np.ascontiguousarray((Dre * S1 * su[None, :]).astype(np.float32).astype(e4))
    dib = np.ascontiguousarray((Dim * S1 * su[None, :]).astype(np.float32).astype(e4))
    dnb = np.ascontiguousarray((-Dim * S1 * su[None, :]).astype(np.float32).astype(e4))
    # stage-2 stacks: quantize N_r first, then build stacks from the
    # quantized planes (so A/B/C/D share bit-identical entries).
    b = np.arange(128, dtype=np.float64)
    y = np.arange(128, dtype=np.float64)
    base = 2.0 * np.pi * np.outer(b, y) / 128.0
    Nq = []
    for r in range(4):
        a = base + (2.0 * np.pi * b * r / H)[:, None]
        m_r = np.cos(a) * scale * S2 * sq[None, :]
        m_i = -np.sin(a) * scale * S2 * sq[None, :]
        Nq.append((m_r.astype(np.float32).astype(e4).astype(np.float32),
                   m_i.astype(np.float32).astype(e4).astype(np.float32)))
    sth = np.empty((128, 12, 2, 128), np.float32)
    for r, (Nr, Ni) in enumerate(Nq):
        if r == 0:
            stacks = [(Nr, Nr), (-Ni, -Ni), (Ni, Ni)]
        elif r == 1:
            stacks = [(Nr, Ni), (-Ni, Nr), (Ni, -Nr)]
        elif r == 2:
            stacks = [(Nr, -Nr), (-Ni, Ni), (Ni, -Ni)]
        else:
            stacks = [(Nr, -Ni), (-Ni, -Nr), (Ni, Nr)]
        for kk, (m0, m1) in enumerate(stacks):
            sth[:, r * 3 + kk, 0, :] = m0
            sth[:, r * 3 + kk, 1, :] = m1
    stb = np.ascontiguousarray(sth.astype(e4))
    return su, sq, drb, dib, dnb, stb


def kernel(predictions, ground_truths, band_weights, band_masks):
    global last_results, last_nc, last_in_maps
    pred = np.ascontiguousarray(np.asarray(predictions, dtype=np.float32))
    gt = np.ascontiguousarray(np.asarray(ground_truths, dtype=np.float32))
    bw = np.asarray(band_weights, dtype=np.float64)
    bm = np.asarray(band_masks, dtype=np.float64)

    wmap = np.einsum('b,bhw->hw', bw, bm)          # shifted coords
    wu = np.fft.ifftshift(wmap)                     # [u_h, v_w] unshifted
    su, sq, drb, dib, dnb, stb = _host_constants()
    # weight tile Wt[y, r, u] = Wu[u, 4y+r] / (su[u]^2 sq[y]^2 S1^2 S2^2)
    yv = np.arange(128)
    wth = np.empty((128, 4, H), np.float64)
    for r in range(4):
        wth[:, r, :] = wu[:, 4 * yv + r].T
    wth /= (su[None, None, :] ** 2) * (sq[:, None, None] ** 2) \
        * (S1 * S1 * S2 * S2)
    bf = ml_dtypes.bfloat16
    wtb = np.ascontiguousarray(wth.astype(np.float32).astype(bf))

    pred_r = pred.reshape(N_CORES, IMGS_PER_CORE, H, H)
    gt_r = gt.reshape(N_CORES, IMGS_PER_CORE, H, H)
    in_maps = [
        {
            "pred": np.ascontiguousarray(pred_r[c]),
            "gt": np.ascontiguousarray(-gt_r[c]),
            "d_r": drb, "d_i": dib, "d_n": dnb,
            "st": stb, "wt": wtb,
        }
        for c in range(N_CORES)
    ]

    nc = BUILD()
    last_nc, last_in_maps = nc, in_maps
    res = run_bass_kernel_spmd(nc, in_maps, core_ids=list(range(N_CORES)))
    last_results = res
    total = np.float64(0.0)
    for r in res.results:
        total += r["out"].astype(np.float64).sum()
    loss = total / float(N * C * H * H)
    return np.float32(loss)
